# revision 11
# baseline (speedup 1.0000x reference)
"""TRN2 Bass kernel for nn_CosFreqEncoding: out = ((x @ W.T) @ cos_basis) / max.

Strategy: data-parallel over batch across 8 NeuronCores. Each core computes
its 512-row shard of both GEMMs in bf16 (fp32 PSUM accumulation, which keeps
rel err ~4e-3, well under the 2e-2 gate), a local max, one scalar
AllReduce(max), then scales and writes its output shard in bf16 (host
converts to fp32 after the gather — the normalization makes any global
positive scaling wash out, and bf16 quantization of values in [-1,1] adds
<2^-9 abs error).

bf16 vs the old fp32r build: half the HBM traffic (24MB -> fully hidden
under compute) and TensorE fast-weight-load (FWL) is enabled, so LDWEIGHTS
(~217ns at fp32r, longer than the 213ns MM-512) stops stalling the PE.

Layouts (host-prepped so no on-chip transposes are needed):
  GEMM1: xfT[f, m] += W.T[l, f].T @ x.T[l, m]   (lhsT = W.T block, rhs = x.T)
  GEMM2: out[m, l2] += xfT[f, m].T @ cos[f, l2] (lhsT = xfT slice, rhs = cos)

The global max is exchanged with ONE AllReduce (the old build used two,
serialized on the single CC stream at ~30us each). gpsimd carries no DMA
work so its collective trigger fires immediately after the last local max.

Self-contained: hardcodes shapes from the problem spec.
"""
import numpy as np
import ml_dtypes

import concourse.bass as bass
import concourse.bass_isa as bass_isa
import concourse.bacc as bacc
import concourse.mybir as mybir
import concourse.tile as tile
import concourse.bass_utils as bass_utils

N_CORES = 8
B, L, F = 4096, 2048, 2074
FP = 2176               # F padded to 17 full 128-tiles
BS = B // N_CORES       # 512 batch rows per core
LT = L // 128           # 16 l-tiles (GEMM1 contraction)
FT = FP // 128          # 17 f-tiles
MT = BS // 128          # 4 m-tiles
CK = L // 512           # 4 output column chunks of 512
F32 = mybir.dt.float32
BF16 = mybir.dt.bfloat16
NP_BF16 = ml_dtypes.bfloat16


def _emit(nc, tc, xT, Wb, cosb, out):
    with (
        tc.tile_pool(name="xp", bufs=1) as xp,
        tc.tile_pool(name="wp", bufs=3) as wp,
        tc.tile_pool(name="xfp", bufs=1) as xfp,
        tc.tile_pool(name="cp", bufs=2) as cp,
        tc.tile_pool(name="op", bufs=1) as op,
        tc.tile_pool(name="sp", bufs=1) as sp,
        tc.tile_pool(name="ps1", bufs=2, space="PSUM") as ps1,
        tc.tile_pool(name="ps2", bufs=6, space="PSUM") as ps2,
        tc.tile_pool(name="dp", bufs=1, space="DRAM") as dp,
    ):
        # DMA issue queues: sync + scalar carry the bulk input streams.
        # gpsimd stays empty so the AllReduce trigger is never queued behind
        # descriptor generation.

        # GEMM1: xfT[f-tile] [128 f, 512 m]; W streamed one 512KB DMA per
        # f-tile column (16 lhsT blocks each). DMA priority order: the first
        # matmul's inputs (wcol0, xt q0) head their queues; the 4.4MB cos
        # prefetch is emitted mid-GEMM1 so the SDMA round-robin does not
        # starve first-needed transfers (front-loading cos pushed the first
        # matmul from ~12us to ~39us).
        ct = [cp.tile([128, FT * 512], BF16, tag="cos", name=f"ct{ci % 2}")
              for ci in range(2)]
        xf = [xfp.tile([128, BS], BF16, name=f"xf{fi}") for fi in range(FT)]
        wc = [wp.tile([128, LT * 128], BF16, tag="w", name=f"w{fi % 3}")
              for fi in range(FT)]

        def wdma(fi):
            q = nc.scalar if fi % 2 == 0 else nc.sync
            q.dma_start(wc[fi][:].rearrange("p (li b) -> p li b", li=LT),
                        Wb[fi])

        xt = xp.tile([128, LT * BS], BF16, name="xt")
        xtv = xt[:].rearrange("p (li b) -> p li b", li=LT)
        wdma(0)
        for qi in range(4):
            (nc.sync if qi % 2 == 0 else nc.scalar).dma_start(
                xtv[:, qi * 4:(qi + 1) * 4], xT[:, qi * 4:(qi + 1) * 4])
        wdma(1)
        for fi in range(FT):
            if fi + 2 < FT:
                wdma(fi + 2)
            if fi == 4:
                nc.gpsimd.dma_start(ct[0][:], cosb[0])
            elif fi == 8:
                nc.gpsimd.dma_start(ct[1][:], cosb[1])
            ps = ps1.tile([128, BS], F32, tag="g1")
            for li in range(LT):
                nc.tensor.matmul(ps[:], wc[fi][:, li * 128:(li + 1) * 128],
                                 xt[:, li * BS:(li + 1) * BS],
                                 start=(li == 0), stop=(li == LT - 1))
            # cast fp32 -> bf16 while copying out of PSUM
            nc.vector.tensor_copy(xf[fi][:], ps[:])

        # GEMM2 + fused local max. Chunk ci's cos block arrives as one
        # 2.2MB DMA prefetched during chunk ci-1 (or GEMM1 for ci<=1).
        ot = [op.tile([128, L], BF16, name=f"ot{mi}") for mi in range(MT)]
        vmaxes = sp.tile([128, MT], F32)
        lm = sp.tile([128, 1], F32)
        for ci in range(CK):
            c = ct[ci % 2]
            # mi outer: each PSUM bank takes 17 back-to-back accumulating
            # matmuls (no per-instruction bank cycling, which triggers HAM
            # re-throttling), and bank mi drains while mi+1 still matmuls.
            for mi in range(MT):
                pst = ps2.tile([128, 512], F32, tag="g2",
                               name=f"ps2_{ci}_{mi}")
                for fi in range(FT):
                    nc.tensor.matmul(
                        pst[:], xf[fi][:, mi * 128:(mi + 1) * 128],
                        c[:, fi * 512:(fi + 1) * 512],
                        start=(fi == 0), stop=(fi == FT - 1))
                # prefetch emitted only after the chunk's LAST matmul: a
                # tile-write is ordered before later-emitted readers, so an
                # earlier emission would feed chunk ci+2's cos to mi>0.
                if mi == MT - 1 and ci + 2 < CK:
                    nc.gpsimd.dma_start(ct[ci % 2][:], cosb[ci + 2])
                nc.vector.reduce_max(vmaxes[:, mi:mi + 1], pst[:],
                                     axis=mybir.AxisListType.X)
                nc.vector.tensor_copy(ot[mi][:, ci * 512:(ci + 1) * 512],
                                      pst[:])
            # running max: after the last chunk only one tiny reduce is
            # left before the AllReduce trigger.
            if ci == 0:
                nc.vector.reduce_max(lm[:], vmaxes[:],
                                     axis=mybir.AxisListType.X)
            else:
                nc.vector.reduce_max(vmaxes[:, 0:1], vmaxes[:],
                                     axis=mybir.AxisListType.X)
                nc.vector.tensor_scalar_max(lm[:], lm[:], vmaxes[:, 0:1])

        # single scalar AllReduce(max); gpsimd queue is nearly empty so the
        # trigger fires right after the final local reduce. The local scalar
        # max is vector XYZW + partition_all_reduce (gpsimd XYZWC is slow).
        lmb = sp.tile([128, 1], F32)
        nc.gpsimd.partition_all_reduce(lmb[:], lm[:], channels=128,
                                       reduce_op=bass_isa.ReduceOp.max)
        cc_in = dp.tile([1], F32, name="ccin")
        cc_out = dp.tile([1], F32, name="ccout")
        nc.gpsimd.dma_start(cc_in[:], lmb[0:1, 0])
        nc.gpsimd.collective_compute(
            "AllReduce", mybir.AluOpType.max,
            replica_groups=[list(range(N_CORES))],
            ins=[cc_in[:]], outs=[cc_out[:]])
        gbc = sp.tile([128, 1], F32)
        nc.gpsimd.dma_start(gbc[:], cc_out[:].partition_broadcast(128))
        rbc = sp.tile([128, 1], F32)
        nc.vector.reciprocal(rbc[:], gbc[:])

        # scale + store; full 4KB-per-partition rows so each write is one
        # 512KB descriptor-friendly DMA.
        for mi in range(MT):
            nc.vector.tensor_scalar_mul(ot[mi][:], ot[mi][:], rbc[:, 0:1])
            (nc.sync if mi % 2 == 0 else nc.scalar).dma_start(
                out[mi * 128:(mi + 1) * 128, :], ot[mi][:])


def _build():
    nc = bacc.Bacc("TRN2", target_bir_lowering=False, debug=False,
                   num_devices=N_CORES)
    xT = nc.dram_tensor("xT", [128, LT, BS], BF16, kind="ExternalInput")
    Wb = nc.dram_tensor("Wb", [FT, 128, LT, 128], BF16, kind="ExternalInput")
    cosb = nc.dram_tensor("cosb", [CK, 128, FT, 512], BF16,
                          kind="ExternalInput")
    out = nc.dram_tensor("out", [BS, L], BF16, kind="ExternalOutput")
    with tile.TileContext(nc) as tc:
        _emit(nc, tc, xT, Wb, cosb, out)
    nc.compile()
    return nc


_cached_nc = None


def _get_nc():
    global _cached_nc
    if _cached_nc is None:
        _cached_nc = _build()
    return _cached_nc


def _prep_inputs(x, W, cos_basis):
    x = np.ascontiguousarray(x, dtype=np.float32)
    W = np.ascontiguousarray(W, dtype=np.float32)
    cos = np.ascontiguousarray(cos_basis, dtype=np.float32)
    # pad freq dim to FP with zeros
    Wp = np.zeros((FP, L), dtype=np.float32)
    Wp[:F] = W
    cosp = np.zeros((FP, L), dtype=np.float32)
    cosp[:F] = cos
    # Wb[fi, p, li, b] = W.T[li*128+p, fi*128+b] = Wp[fi*128+b, li*128+p]
    Wb = np.ascontiguousarray(
        Wp.reshape(FT, 128, LT, 128).transpose(0, 3, 2, 1).astype(NP_BF16))
    # cosb[ci, p, fi, n] = cosp[fi*128+p, ci*512+n]
    cosb = np.ascontiguousarray(
        cosp.reshape(FT, 128, CK, 512).transpose(2, 1, 0, 3).astype(NP_BF16))
    # xT[p, li, m] = x_shard[m, li*128+p]
    xTs = []
    for i in range(N_CORES):
        sh = x[i * BS:(i + 1) * BS].reshape(BS, LT, 128)
        xTs.append(np.ascontiguousarray(
            sh.transpose(2, 1, 0).astype(NP_BF16)))
    return xTs, Wb, cosb


def kernel(x, W, cos_basis, _trace=False, _trace_kwargs=None):
    xTs, Wb, cosb = _prep_inputs(x, W, cos_basis)
    nc = _get_nc()
    in_maps = [{"xT": xTs[i], "Wb": Wb, "cosb": cosb} for i in range(N_CORES)]
    res = bass_utils.run_bass_kernel_spmd(
        nc, in_maps, core_ids=list(range(N_CORES)), trace=_trace,
        **(_trace_kwargs or {}))
    out = np.concatenate(
        [res.results[i]["out"].astype(np.float32) for i in range(N_CORES)],
        axis=0)
    if _trace:
        kernel.last_result = res
    return out


# revision 13
# speedup vs baseline: 1.1237x; 1.1237x over previous
"""TRN2 Bass kernel for nn_CosFreqEncoding: out = ((x @ W.T) @ cos_basis) / max.

Strategy: data-parallel over batch across 8 NeuronCores. Each core computes
its 512-row shard of both GEMMs in bf16 (fp32 PSUM accumulation, which keeps
rel err ~4e-3, well under the 2e-2 gate), a local max, one scalar
AllReduce(max), then scales and writes its output shard in bf16 (host
converts to fp32 after the gather — the normalization makes any global
positive scaling wash out, and bf16 quantization of values in [-1,1] adds
<2^-9 abs error).

bf16 vs the old fp32r build: half the HBM traffic (24MB -> fully hidden
under compute) and TensorE fast-weight-load (FWL) is enabled, so LDWEIGHTS
(~217ns at fp32r, longer than the 213ns MM-512) stops stalling the PE.

Layouts (host-prepped so no on-chip transposes are needed):
  GEMM1: xfT[f, m] += W.T[l, f].T @ x.T[l, m]   (lhsT = W.T block, rhs = x.T)
  GEMM2: out[m, l2] += xfT[f, m].T @ cos[f, l2] (lhsT = xfT slice, rhs = cos)

The global max is exchanged with ONE AllReduce (the old build used two,
serialized on the single CC stream at ~30us each). gpsimd carries no DMA
work so its collective trigger fires immediately after the last local max.

Self-contained: hardcodes shapes from the problem spec.
"""
import numpy as np
import ml_dtypes

import concourse.bass as bass
import concourse.bass_isa as bass_isa
import concourse.bacc as bacc
import concourse.mybir as mybir
import concourse.tile as tile
import concourse.bass_utils as bass_utils

N_CORES = 8
B, L, F = 4096, 2048, 2074
FP = 2176               # F padded to 17 full 128-tiles
BS = B // N_CORES       # 512 batch rows per core
LT = L // 128           # 16 l-tiles (GEMM1 contraction)
FT = FP // 128          # 17 f-tiles
MT = BS // 128          # 4 m-tiles
CK = L // 512           # 4 output column chunks of 512
F32 = mybir.dt.float32
BF16 = mybir.dt.bfloat16
NP_BF16 = ml_dtypes.bfloat16


def _emit(nc, tc, xT, Wb, cosb, out):
    with (
        tc.tile_pool(name="xp", bufs=1) as xp,
        tc.tile_pool(name="wp", bufs=3) as wp,
        tc.tile_pool(name="xfp", bufs=1) as xfp,
        tc.tile_pool(name="cp", bufs=2) as cp,
        tc.tile_pool(name="op", bufs=1) as op,
        tc.tile_pool(name="sp", bufs=1) as sp,
        tc.tile_pool(name="ps1", bufs=2, space="PSUM") as ps1,
        tc.tile_pool(name="ps2", bufs=6, space="PSUM") as ps2,
        tc.tile_pool(name="dp", bufs=1, space="DRAM") as dp,
    ):
        # DMA issue queues: sync + scalar carry the bulk input streams.
        # gpsimd stays empty so the AllReduce trigger is never queued behind
        # descriptor generation.

        # GEMM1: xfT[f-tile] [128 f, 512 m]; W streamed one 512KB DMA per
        # f-tile column (16 lhsT blocks each). DMA priority order: the first
        # matmul's inputs (wcol0, xt q0) head their queues; the 4.4MB cos
        # prefetch is emitted mid-GEMM1 so the SDMA round-robin does not
        # starve first-needed transfers (front-loading cos pushed the first
        # matmul from ~12us to ~39us).
        ct = [cp.tile([128, FT * 512], BF16, tag="cos", name=f"ct{ci % 2}")
              for ci in range(2)]
        xf = [xfp.tile([128, BS], BF16, name=f"xf{fi}") for fi in range(FT)]
        wc = [wp.tile([128, LT * 128], BF16, tag="w", name=f"w{fi % 3}")
              for fi in range(FT)]

        def wdma(fi):
            q = nc.scalar if fi % 2 == 0 else nc.sync
            q.dma_start(wc[fi][:].rearrange("p (li b) -> p li b", li=LT),
                        Wb[fi])

        xt = xp.tile([128, LT * BS], BF16, name="xt")
        xtv = xt[:].rearrange("p (li b) -> p li b", li=LT)
        wdma(0)
        for qi in range(4):
            (nc.sync if qi % 2 == 0 else nc.scalar).dma_start(
                xtv[:, qi * 4:(qi + 1) * 4], xT[:, qi * 4:(qi + 1) * 4])
        wdma(1)
        # cos loads wait on a joiner DMA that reads xf[8] (done ~mid-GEMM1):
        # gpsimd's DGE queue is FIFO, so the joiner at its head delays ct0/
        # ct1 until the W stream no longer needs the HBM bandwidth. (An
        # early-emitted gpsimd DMA executes immediately regardless of
        # program position - gpsimd has nothing else queued.)
        joiner = dp.tile([1, 1], BF16, name="joiner")
        for fi in range(FT):
            if fi + 2 < FT:
                wdma(fi + 2)
            ps = ps1.tile([128, BS], F32, tag="g1")
            for li in range(LT):
                nc.tensor.matmul(ps[:], wc[fi][:, li * 128:(li + 1) * 128],
                                 xt[:, li * BS:(li + 1) * BS],
                                 start=(li == 0), stop=(li == LT - 1))
            # cast fp32 -> bf16 while copying out of PSUM
            nc.vector.tensor_copy(xf[fi][:], ps[:])
            if fi == 8:
                nc.gpsimd.dma_start(joiner[:], xf[8][0:1, 0:1])
                nc.gpsimd.dma_start(ct[0][:], cosb[0])
                nc.gpsimd.dma_start(ct[1][:], cosb[1])

        # GEMM2 + fused local max. Chunk ci's cos block arrives as one
        # 2.2MB DMA prefetched during chunk ci-1 (or GEMM1 for ci<=1).
        # Two-stage AllReduce(max): stage 1 covers chunk 0 and is triggered
        # right after it, hiding its ~35us (trigger floor + launch-skew
        # absorption) under chunks 1-3. Stage 2 covers chunks 1-3 and only
        # pays the ~20us protocol floor at the tail, since stage 1 already
        # absorbed the inter-core skew on the shared CC stream.
        def armax(lmx, tag):
            lmb = sp.tile([128, 1], F32, name=f"lmb_{tag}")
            nc.gpsimd.partition_all_reduce(lmb[:], lmx[:], channels=128,
                                           reduce_op=bass_isa.ReduceOp.max)
            cc_in = dp.tile([1], F32, name=f"ccin_{tag}")
            cc_out = dp.tile([1], F32, name=f"ccout_{tag}")
            nc.gpsimd.dma_start(cc_in[:], lmb[0:1, 0])
            nc.gpsimd.collective_compute(
                "AllReduce", mybir.AluOpType.max,
                replica_groups=[list(range(N_CORES))],
                ins=[cc_in[:]], outs=[cc_out[:]])
            return cc_out

        ot = [op.tile([128, L], BF16, name=f"ot{mi}") for mi in range(MT)]
        vmaxes = sp.tile([128, MT], F32)
        lm = sp.tile([128, 1], F32)
        lm2 = sp.tile([128, 1], F32)
        for ci in range(CK):
            c = ct[ci % 2]
            # mi outer: each PSUM bank takes 17 back-to-back accumulating
            # matmuls (no per-instruction bank cycling, which triggers HAM
            # re-throttling), and bank mi drains while mi+1 still matmuls.
            for mi in range(MT):
                pst = ps2.tile([128, 512], F32, tag="g2",
                               name=f"ps2_{ci}_{mi}")
                for fi in range(FT):
                    nc.tensor.matmul(
                        pst[:], xf[fi][:, mi * 128:(mi + 1) * 128],
                        c[:, fi * 512:(fi + 1) * 512],
                        start=(fi == 0), stop=(fi == FT - 1))
                # cos reload emitted only after the chunk's LAST matmul (a
                # tile-write is ordered before later-emitted readers), on
                # the sync/scalar queues which are idle during GEMM2.
                if mi == MT - 1 and ci + 2 < CK:
                    (nc.sync if ci == 0 else nc.scalar).dma_start(
                        ct[ci % 2][:], cosb[ci + 2])
                nc.vector.reduce_max(vmaxes[:, mi:mi + 1], pst[:],
                                     axis=mybir.AxisListType.X)
                nc.vector.tensor_copy(ot[mi][:, ci * 512:(ci + 1) * 512],
                                      pst[:])
            # per-chunk running max, kept in [128,1] so the pre-trigger
            # chain after the last chunk is one vector op + gpsimd reduce.
            if ci == 0:
                nc.vector.reduce_max(lm[:], vmaxes[:],
                                     axis=mybir.AxisListType.X)
                cc1_out = armax(lm, "s1")
            elif ci == 1:
                nc.vector.reduce_max(lm2[:], vmaxes[:],
                                     axis=mybir.AxisListType.X)
            else:
                nc.vector.reduce_max(vmaxes[:, 0:1], vmaxes[:],
                                     axis=mybir.AxisListType.X)
                nc.vector.tensor_scalar_max(lm2[:], lm2[:], vmaxes[:, 0:1])

        cc2_out = armax(lm2, "s2")
        gbc1 = sp.tile([128, 1], F32)
        nc.gpsimd.dma_start(gbc1[:], cc1_out[:].partition_broadcast(128))
        gbc2 = sp.tile([128, 1], F32)
        nc.gpsimd.dma_start(gbc2[:], cc2_out[:].partition_broadcast(128))
        gbc = sp.tile([128, 1], F32)
        nc.vector.tensor_scalar_max(gbc[:], gbc1[:], gbc2[:, 0:1])
        rbc = sp.tile([128, 1], F32)
        nc.vector.reciprocal(rbc[:], gbc[:])

        # scale + store; full 4KB-per-partition rows so each write is one
        # 512KB descriptor-friendly DMA.
        for mi in range(MT):
            nc.vector.tensor_scalar_mul(ot[mi][:], ot[mi][:], rbc[:, 0:1])
            (nc.sync if mi % 2 == 0 else nc.scalar).dma_start(
                out[mi * 128:(mi + 1) * 128, :], ot[mi][:])


def _build():
    nc = bacc.Bacc("TRN2", target_bir_lowering=False, debug=False,
                   num_devices=N_CORES)
    xT = nc.dram_tensor("xT", [128, LT, BS], BF16, kind="ExternalInput")
    Wb = nc.dram_tensor("Wb", [FT, 128, LT, 128], BF16, kind="ExternalInput")
    cosb = nc.dram_tensor("cosb", [CK, 128, FT, 512], BF16,
                          kind="ExternalInput")
    out = nc.dram_tensor("out", [BS, L], BF16, kind="ExternalOutput")
    with tile.TileContext(nc) as tc:
        _emit(nc, tc, xT, Wb, cosb, out)
    nc.compile()
    return nc


_cached_nc = None


def _get_nc():
    global _cached_nc
    if _cached_nc is None:
        _cached_nc = _build()
    return _cached_nc


def _prep_inputs(x, W, cos_basis):
    x = np.ascontiguousarray(x, dtype=np.float32)
    W = np.ascontiguousarray(W, dtype=np.float32)
    cos = np.ascontiguousarray(cos_basis, dtype=np.float32)
    # pad freq dim to FP with zeros
    Wp = np.zeros((FP, L), dtype=np.float32)
    Wp[:F] = W
    cosp = np.zeros((FP, L), dtype=np.float32)
    cosp[:F] = cos
    # Wb[fi, p, li, b] = W.T[li*128+p, fi*128+b] = Wp[fi*128+b, li*128+p]
    Wb = np.ascontiguousarray(
        Wp.reshape(FT, 128, LT, 128).transpose(0, 3, 2, 1).astype(NP_BF16))
    # cosb[ci, p, fi, n] = cosp[fi*128+p, ci*512+n]
    cosb = np.ascontiguousarray(
        cosp.reshape(FT, 128, CK, 512).transpose(2, 1, 0, 3).astype(NP_BF16))
    # xT[p, li, m] = x_shard[m, li*128+p]
    xTs = []
    for i in range(N_CORES):
        sh = x[i * BS:(i + 1) * BS].reshape(BS, LT, 128)
        xTs.append(np.ascontiguousarray(
            sh.transpose(2, 1, 0).astype(NP_BF16)))
    return xTs, Wb, cosb


def kernel(x, W, cos_basis, _trace=False, _trace_kwargs=None):
    xTs, Wb, cosb = _prep_inputs(x, W, cos_basis)
    nc = _get_nc()
    in_maps = [{"xT": xTs[i], "Wb": Wb, "cosb": cosb} for i in range(N_CORES)]
    res = bass_utils.run_bass_kernel_spmd(
        nc, in_maps, core_ids=list(range(N_CORES)), trace=_trace,
        **(_trace_kwargs or {}))
    out = np.concatenate(
        [res.results[i]["out"].astype(np.float32) for i in range(N_CORES)],
        axis=0)
    if _trace:
        kernel.last_result = res
    return out


# revision 15
# speedup vs baseline: 1.2623x; 1.1233x over previous
"""TRN2 Bass kernel for nn_CosFreqEncoding: out = ((x @ W.T) @ cos_basis) / max.

Strategy: data-parallel over batch across 8 NeuronCores. Each core computes
its 512-row shard of both GEMMs in bf16 (fp32 PSUM accumulation, which keeps
rel err ~4e-3, well under the 2e-2 gate), a local max, one scalar
AllReduce(max), then scales and writes its output shard in bf16 (host
converts to fp32 after the gather — the normalization makes any global
positive scaling wash out, and bf16 quantization of values in [-1,1] adds
<2^-9 abs error).

bf16 vs the old fp32r build: half the HBM traffic (24MB -> fully hidden
under compute) and TensorE fast-weight-load (FWL) is enabled, so LDWEIGHTS
(~217ns at fp32r, longer than the 213ns MM-512) stops stalling the PE.

Layouts (host-prepped so no on-chip transposes are needed):
  GEMM1: xfT[f, m] += W.T[l, f].T @ x.T[l, m]   (lhsT = W.T block, rhs = x.T)
  GEMM2: out[m, l2] += xfT[f, m].T @ cos[f, l2] (lhsT = xfT slice, rhs = cos)

The global max is exchanged with ONE AllReduce (the old build used two,
serialized on the single CC stream at ~30us each). gpsimd carries no DMA
work so its collective trigger fires immediately after the last local max.

Self-contained: hardcodes shapes from the problem spec.
"""
import numpy as np
import ml_dtypes

import concourse.bass as bass
import concourse.bass_isa as bass_isa
import concourse.bacc as bacc
import concourse.mybir as mybir
import concourse.tile as tile
import concourse.bass_utils as bass_utils

N_CORES = 8
B, L, F = 4096, 2048, 2074
FP = 2176               # F padded to 17 full 128-tiles
BS = B // N_CORES       # 512 batch rows per core
LT = L // 128           # 16 l-tiles (GEMM1 contraction)
FT = FP // 128          # 17 f-tiles
MT = BS // 128          # 4 m-tiles
CK = L // 512           # 4 output column chunks of 512
F32 = mybir.dt.float32
BF16 = mybir.dt.bfloat16
NP_BF16 = ml_dtypes.bfloat16


def _emit(nc, tc, xT, Wb, cosb, out):
    with (
        tc.tile_pool(name="xp", bufs=1) as xp,
        tc.tile_pool(name="wp", bufs=3) as wp,
        tc.tile_pool(name="xfp", bufs=1) as xfp,
        tc.tile_pool(name="cp", bufs=2) as cp,
        tc.tile_pool(name="op", bufs=1) as op,
        tc.tile_pool(name="sp", bufs=1) as sp,
        tc.tile_pool(name="ps1", bufs=2, space="PSUM") as ps1,
        tc.tile_pool(name="ps2", bufs=6, space="PSUM") as ps2,
        tc.tile_pool(name="dp", bufs=1, space="DRAM") as dp,
    ):
        # DMA issue queues: sync + scalar carry the bulk input streams.
        # gpsimd stays empty so the AllReduce trigger is never queued behind
        # descriptor generation.

        # GEMM1: xfT[f-tile] [128 f, 512 m]; W streamed one 512KB DMA per
        # f-tile column (16 lhsT blocks each). DMA priority order: the first
        # matmul's inputs (wcol0, xt q0) head their queues; the 4.4MB cos
        # prefetch is emitted mid-GEMM1 so the SDMA round-robin does not
        # starve first-needed transfers (front-loading cos pushed the first
        # matmul from ~12us to ~39us).
        ct = [cp.tile([128, FT * 512], BF16, tag="cos", name=f"ct{ci % 2}")
              for ci in range(2)]
        xf = [xfp.tile([128, BS], BF16, name=f"xf{fi}") for fi in range(FT)]
        wc = [wp.tile([128, LT * 128], BF16, tag="w", name=f"w{fi % 3}")
              for fi in range(FT)]

        def wdma(fi):
            q = nc.scalar if fi % 2 == 0 else nc.sync
            q.dma_start(wc[fi][:].rearrange("p (li b) -> p li b", li=LT),
                        Wb[fi])

        xt = xp.tile([128, LT * BS], BF16, name="xt")
        xtv = xt[:].rearrange("p (li b) -> p li b", li=LT)
        wdma(0)
        for qi in range(4):
            (nc.sync if qi % 2 == 0 else nc.scalar).dma_start(
                xtv[:, qi * 4:(qi + 1) * 4], xT[:, qi * 4:(qi + 1) * 4])
        wdma(1)
        # cos loads wait on a joiner DMA that reads xf[8] (done ~mid-GEMM1):
        # gpsimd's DGE queue is FIFO, so the joiner at its head delays ct0/
        # ct1 until the W stream no longer needs the HBM bandwidth. (An
        # early-emitted gpsimd DMA executes immediately regardless of
        # program position - gpsimd has nothing else queued.)
        joiner = dp.tile([1, 1], BF16, name="joiner")
        for fi in range(FT):
            if fi + 2 < FT:
                wdma(fi + 2)
            ps = ps1.tile([128, BS], F32, tag="g1")
            for li in range(LT):
                nc.tensor.matmul(ps[:], wc[fi][:, li * 128:(li + 1) * 128],
                                 xt[:, li * BS:(li + 1) * BS],
                                 start=(li == 0), stop=(li == LT - 1))
            # cast fp32 -> bf16 while copying out of PSUM
            nc.vector.tensor_copy(xf[fi][:], ps[:])
            if fi == 8:
                nc.gpsimd.dma_start(joiner[:], xf[8][0:1, 0:1])
                nc.gpsimd.dma_start(ct[0][:], cosb[0])
                nc.gpsimd.dma_start(ct[1][:], cosb[1])

        # GEMM2 + fused local max. Chunk ci's cos block arrives as one
        # 2.2MB DMA prefetched during chunk ci-1 (or GEMM1 for ci<=1).
        # Two-stage AllReduce(max): stage 1 covers chunk 0 and is triggered
        # right after it, hiding its ~35us (trigger floor + launch-skew
        # absorption) under chunks 1-3. Stage 2 covers chunks 1-3 and only
        # pays the ~20us protocol floor at the tail, since stage 1 already
        # absorbed the inter-core skew on the shared CC stream.
        def armax(lmx, tag):
            lmb = sp.tile([128, 1], F32, name=f"lmb_{tag}")
            nc.gpsimd.partition_all_reduce(lmb[:], lmx[:], channels=128,
                                           reduce_op=bass_isa.ReduceOp.max)
            cc_in = dp.tile([1], F32, name=f"ccin_{tag}")
            cc_out = dp.tile([1], F32, name=f"ccout_{tag}")
            nc.gpsimd.dma_start(cc_in[:], lmb[0:1, 0])
            nc.gpsimd.collective_compute(
                "AllReduce", mybir.AluOpType.max,
                replica_groups=[list(range(N_CORES))],
                ins=[cc_in[:]], outs=[cc_out[:]])
            return cc_out

        ot = [op.tile([128, L], BF16, name=f"ot{mi}") for mi in range(MT)]
        vmaxes = sp.tile([128, MT], F32)
        lm = sp.tile([128, 1], F32)
        lm2 = sp.tile([128, 1], F32)
        for ci in range(CK):
            c = ct[ci % 2]
            # mi outer: each PSUM bank takes 17 back-to-back accumulating
            # matmuls (no per-instruction bank cycling, which triggers HAM
            # re-throttling), and bank mi drains while mi+1 still matmuls.
            for mi in range(MT):
                pst = ps2.tile([128, 512], F32, tag="g2",
                               name=f"ps2_{ci}_{mi}")
                for fi in range(FT):
                    nc.tensor.matmul(
                        pst[:], xf[fi][:, mi * 128:(mi + 1) * 128],
                        c[:, fi * 512:(fi + 1) * 512],
                        start=(fi == 0), stop=(fi == FT - 1))
                # cos reload emitted only after the chunk's LAST matmul (a
                # tile-write is ordered before later-emitted readers), on
                # the sync/scalar queues which are idle during GEMM2.
                if mi == MT - 1 and ci + 2 < CK:
                    (nc.sync if ci == 0 else nc.scalar).dma_start(
                        ct[ci % 2][:], cosb[ci + 2])
                # per-mi running max keeps the post-last-matmul chain to a
                # single tensor_scalar_max before the stage-2 trigger.
                nc.vector.reduce_max(vmaxes[:, mi:mi + 1], pst[:],
                                     axis=mybir.AxisListType.X)
                tgt = lm if ci == 0 else lm2
                if mi == 0 and ci in (0, 1):
                    nc.vector.tensor_copy(tgt[:], vmaxes[:, 0:1])
                else:
                    nc.vector.tensor_scalar_max(tgt[:], tgt[:],
                                                vmaxes[:, mi:mi + 1])
                nc.vector.tensor_copy(ot[mi][:, ci * 512:(ci + 1) * 512],
                                      pst[:])
            if ci == 0:
                cc1_out = armax(lm, "s1")

        cc2_out = armax(lm2, "s2")
        # broadcast-read the two AllReduce results on the idle HWDGE queues
        # (gpsimd's SWDGE adds ~1.5us each), combine, reciprocal.
        gbc1 = sp.tile([128, 1], F32)
        nc.sync.dma_start(gbc1[:], cc1_out[:].partition_broadcast(128))
        gbc2 = sp.tile([128, 1], F32)
        nc.scalar.dma_start(gbc2[:], cc2_out[:].partition_broadcast(128))
        gbc = sp.tile([128, 1], F32)
        nc.vector.tensor_scalar_max(gbc[:], gbc1[:], gbc2[:, 0:1])
        rbc = sp.tile([128, 1], F32)
        nc.vector.reciprocal(rbc[:], gbc[:])

        # scale + store, half-tiles so the first DMA starts after ~0.3us of
        # vector work and write DMAs pipeline across both HWDGE queues.
        for mi in range(MT):
            for h in range(2):
                sl = slice(h * 1024, (h + 1) * 1024)
                nc.vector.tensor_scalar_mul(ot[mi][:, sl], ot[mi][:, sl],
                                            rbc[:, 0:1])
                (nc.sync if (mi * 2 + h) % 2 == 0 else nc.scalar).dma_start(
                    out[mi * 128:(mi + 1) * 128, sl], ot[mi][:, sl])


def _build():
    nc = bacc.Bacc("TRN2", target_bir_lowering=False, debug=False,
                   num_devices=N_CORES)
    xT = nc.dram_tensor("xT", [128, LT, BS], BF16, kind="ExternalInput")
    Wb = nc.dram_tensor("Wb", [FT, 128, LT, 128], BF16, kind="ExternalInput")
    cosb = nc.dram_tensor("cosb", [CK, 128, FT, 512], BF16,
                          kind="ExternalInput")
    out = nc.dram_tensor("out", [BS, L], BF16, kind="ExternalOutput")
    with tile.TileContext(nc) as tc:
        _emit(nc, tc, xT, Wb, cosb, out)
    nc.compile()
    return nc


_cached_nc = None


def _get_nc():
    global _cached_nc
    if _cached_nc is None:
        _cached_nc = _build()
    return _cached_nc


def _prep_inputs(x, W, cos_basis):
    x = np.ascontiguousarray(x, dtype=np.float32)
    W = np.ascontiguousarray(W, dtype=np.float32)
    cos = np.ascontiguousarray(cos_basis, dtype=np.float32)
    # pad freq dim to FP with zeros
    Wp = np.zeros((FP, L), dtype=np.float32)
    Wp[:F] = W
    cosp = np.zeros((FP, L), dtype=np.float32)
    cosp[:F] = cos
    # Wb[fi, p, li, b] = W.T[li*128+p, fi*128+b] = Wp[fi*128+b, li*128+p]
    Wb = np.ascontiguousarray(
        Wp.reshape(FT, 128, LT, 128).transpose(0, 3, 2, 1).astype(NP_BF16))
    # cosb[ci, p, fi, n] = cosp[fi*128+p, ci*512+n]
    cosb = np.ascontiguousarray(
        cosp.reshape(FT, 128, CK, 512).transpose(2, 1, 0, 3).astype(NP_BF16))
    # xT[p, li, m] = x_shard[m, li*128+p]
    xTs = []
    for i in range(N_CORES):
        sh = x[i * BS:(i + 1) * BS].reshape(BS, LT, 128)
        xTs.append(np.ascontiguousarray(
            sh.transpose(2, 1, 0).astype(NP_BF16)))
    return xTs, Wb, cosb


_out_names = ["out"]


def _in_maps(x, W, cos_basis):
    xTs, Wb, cosb = _prep_inputs(x, W, cos_basis)
    return [{"xT": xTs[i], "Wb": Wb, "cosb": cosb} for i in range(N_CORES)]


def _post(results):
    return np.concatenate(
        [results[i]["out"].astype(np.float32) for i in range(N_CORES)],
        axis=0)


def kernel(x, W, cos_basis, _trace=False, _trace_kwargs=None):
    in_maps = _in_maps(x, W, cos_basis)
    nc = _get_nc()
    res = bass_utils.run_bass_kernel_spmd(
        nc, in_maps, core_ids=list(range(N_CORES)), trace=_trace,
        **(_trace_kwargs or {}))
    out = _post(res.results)
    if _trace:
        kernel.last_result = res
    return out


# revision 16
# speedup vs baseline: 1.2637x; 1.0011x over previous
"""Factored TRN2 kernel: out = x @ M with M = W.T @ cos_basis, then /max.

Each core computes M[:, s*256:(s+1)*256] (its 256-column shard of M, no
redundancy) and then out.T[s-shard, :] = (x @ M[:, shard]).T for the FULL
batch. Column sharding means x is replicated (16MB bf16 per core) but total
PE work drops 28% vs the direct two-GEMM form: phase 1 is 272 MM-256 pairs
(1.14e9 MAC) instead of 272 MM-512, phase 2 is 256 MM-512 (2.15e9 MAC).

Phase 1 (M-form): psum[l-tile, 256] += W-block[f,l].T @ cos_shard[f, 256]
  lhsT = W blocks with partition=f (host: Wp[li, p(f), fi... see prep)
  -> M tiles [128 l, 256 l2] in SBUF bf16: exactly the phase-2 lhsT layout.
Phase 2: psum[l2-part, 512 m] += M-block[l, l2].T @ xT[l, m]
  out.T tiles written per (l2p, m-chunk); host transposes/concats (free).

Max: same two-stage scalar AllReduce(max) as the direct kernel.
"""
import numpy as np
import ml_dtypes

import concourse.bass as bass
import concourse.bass_isa as bass_isa
import concourse.bacc as bacc
import concourse.mybir as mybir
import concourse.tile as tile
import concourse.bass_utils as bass_utils

N_CORES = 8
B, L, F = 4096, 2048, 2074
FP = 2176               # F padded to 17 full 128-tiles
CS = L // N_CORES       # 256 M-columns per core
LT = L // 128           # 16 l-tiles
FT = FP // 128          # 17 f-tiles
MC = 4                  # m-chunks of 1024 in phase 2
MW = B // MC            # 1024 batch columns per chunk
F32 = mybir.dt.float32
BF16 = mybir.dt.bfloat16
NP_BF16 = ml_dtypes.bfloat16


def _emit(nc, tc, Wl, cs, xTf, outT):
    with (
        tc.tile_pool(name="wp", bufs=3) as wp,
        tc.tile_pool(name="csp", bufs=1) as csp,
        tc.tile_pool(name="mp", bufs=1) as mp,
        tc.tile_pool(name="xp", bufs=2) as xp,
        tc.tile_pool(name="op", bufs=1) as op,
        tc.tile_pool(name="sp", bufs=1) as sp,
        tc.tile_pool(name="ps1", bufs=2, space="PSUM") as ps1,
        tc.tile_pool(name="ps2", bufs=3, space="PSUM") as ps2,
        tc.tile_pool(name="dp", bufs=1, space="DRAM") as dp,
    ):
        def armax(lmx, tag):
            lmb = sp.tile([128, 1], F32, name=f"lmb_{tag}")
            nc.gpsimd.partition_all_reduce(lmb[:], lmx[:], channels=128,
                                           reduce_op=bass_isa.ReduceOp.max)
            cc_in = dp.tile([1], F32, name=f"ccin_{tag}")
            cc_out = dp.tile([1], F32, name=f"ccout_{tag}")
            nc.gpsimd.dma_start(cc_in[:], lmb[0:1, 0])
            nc.gpsimd.collective_compute(
                "AllReduce", mybir.AluOpType.max,
                replica_groups=[list(range(N_CORES))],
                ins=[cc_in[:]], outs=[cc_out[:]])
            return cc_out

        # ---- phase 1: M[:, shard] = W.T @ cos[:, shard] ----
        cst = csp.tile([128, FT * CS], BF16, name="cst")
        nc.scalar.dma_start(
            cst[:].rearrange("p (fi n) -> p fi n", fi=FT), cs[:])
        wl = [wp.tile([128, FT * 128], BF16, tag="w", name=f"w{li % 3}")
              for li in range(LT)]

        def wdma(li):
            q = nc.sync if li % 2 == 0 else nc.scalar
            q.dma_start(wl[li][:].rearrange("p (fi b) -> p fi b", fi=FT),
                        Wl[li])

        wdma(0)
        wdma(1)
        mt = [mp.tile([128, CS], BF16, name=f"m{li}") for li in range(LT)]
        joiner = dp.tile([1, 1], BF16, name="joiner")
        for li in range(LT):
            if li + 2 < LT:
                wdma(li + 2)
            ps = ps1.tile([128, CS], F32, tag="m")
            for fi in range(FT):
                nc.tensor.matmul(ps[:], wl[li][:, fi * 128:(fi + 1) * 128],
                                 cst[:, fi * CS:(fi + 1) * CS],
                                 start=(fi == 0), stop=(fi == FT - 1))
            nc.vector.tensor_copy(mt[li][:], ps[:])
            if li == 7:
                # x stream begins once W no longer needs the bandwidth
                nc.gpsimd.dma_start(joiner[:], mt[7][0:1, 0:1])

        # ---- phase 2: out.T[l2p, m] += M-block.T @ xT, streamed per
        # 1024-column m-chunk (4MB bf16 each, 2KB lines) ----
        xc = [xp.tile([128, LT * MW], BF16, tag="x", name=f"xc{c % 2}")
              for c in range(MC)]
        nc.gpsimd.dma_start(xc[0][:].rearrange("p (li m) -> p li m", li=LT),
                            xTf[0])
        nc.gpsimd.dma_start(xc[1][:].rearrange("p (li m) -> p li m", li=LT),
                            xTf[1])

        otp = [op.tile([128, B], BF16, name=f"otp{l2p}") for l2p in range(2)]
        vmaxes = sp.tile([128, 2], F32)
        lm = sp.tile([128, 1], F32)
        lm2 = sp.tile([128, 1], F32)
        for mc in range(MC):
            xcur = xc[mc % 2]
            for l2p in range(2):
                pst = ps2.tile([128, MW], F32, tag="o",
                               name=f"ps2_{mc}_{l2p}")
                for li in range(LT):
                    nc.tensor.matmul(
                        pst[:, 0:512],
                        mt[li][:, l2p * 128:(l2p + 1) * 128],
                        xcur[:, li * MW:li * MW + 512],
                        start=(li == 0), stop=(li == LT - 1))
                for li in range(LT):
                    nc.tensor.matmul(
                        pst[:, 512:MW],
                        mt[li][:, l2p * 128:(l2p + 1) * 128],
                        xcur[:, li * MW + 512:(li + 1) * MW],
                        start=(li == 0), stop=(li == LT - 1))
                if l2p == 1 and mc + 2 < MC:
                    (nc.sync if mc == 0 else nc.scalar).dma_start(
                        xc[mc % 2][:].rearrange("p (li m) -> p li m", li=LT),
                        xTf[mc + 2])
                nc.vector.reduce_max(vmaxes[:, l2p:l2p + 1], pst[:],
                                     axis=mybir.AxisListType.X)
                nc.vector.tensor_copy(
                    otp[l2p][:, mc * MW:(mc + 1) * MW], pst[:])
            if mc == 0:
                nc.vector.reduce_max(lm[:], vmaxes[:],
                                     axis=mybir.AxisListType.X)
                cc1_out = armax(lm, "s1")
            elif mc == 1:
                nc.vector.reduce_max(lm2[:], vmaxes[:],
                                     axis=mybir.AxisListType.X)
            else:
                nc.vector.reduce_max(vmaxes[:, 0:1], vmaxes[:],
                                     axis=mybir.AxisListType.X)
                nc.vector.tensor_scalar_max(lm2[:], lm2[:], vmaxes[:, 0:1])

        cc2_out = armax(lm2, "s2")
        gbc1 = sp.tile([128, 1], F32)
        nc.gpsimd.dma_start(gbc1[:], cc1_out[:].partition_broadcast(128))
        gbc2 = sp.tile([128, 1], F32)
        nc.gpsimd.dma_start(gbc2[:], cc2_out[:].partition_broadcast(128))
        gbc = sp.tile([128, 1], F32)
        nc.vector.tensor_scalar_max(gbc[:], gbc1[:], gbc2[:, 0:1])
        rbc = sp.tile([128, 1], F32)
        nc.vector.reciprocal(rbc[:], gbc[:])

        for l2p in range(2):
            nc.vector.tensor_scalar_mul(otp[l2p][:], otp[l2p][:],
                                        rbc[:, 0:1])
            (nc.sync if l2p == 0 else nc.scalar).dma_start(
                outT[l2p], otp[l2p][:])


def _build():
    nc = bacc.Bacc("TRN2", target_bir_lowering=False, debug=False,
                   num_devices=N_CORES)
    # Wl[li, p(f in tile), fi, b(l in tile)] = Wp[fi*128+p, li*128+b]
    Wl = nc.dram_tensor("Wl", [LT, 128, FT, 128], BF16, kind="ExternalInput")
    # cs[p(f in tile), fi, n] = cosp[fi*128+p, shard_cols[n]]
    cs = nc.dram_tensor("cs", [128, FT, CS], BF16, kind="ExternalInput")
    # xTf[mc, p(l in tile), li, m] = x[mc*1024+m, li*128+p]
    xTf = nc.dram_tensor("xTf", [MC, 128, LT, MW], BF16,
                         kind="ExternalInput")
    outT = nc.dram_tensor("outT", [2, 128, B], BF16, kind="ExternalOutput")
    with tile.TileContext(nc) as tc:
        _emit(nc, tc, Wl, cs, xTf, outT)
    nc.compile()
    return nc


_cached_nc = None


def _get_nc():
    global _cached_nc
    if _cached_nc is None:
        _cached_nc = _build()
    return _cached_nc


def _prep_inputs(x, W, cos_basis):
    x = np.ascontiguousarray(x, dtype=np.float32)
    W = np.ascontiguousarray(W, dtype=np.float32)
    cos = np.ascontiguousarray(cos_basis, dtype=np.float32)
    Wp = np.zeros((FP, L), dtype=np.float32)
    Wp[:F] = W
    cosp = np.zeros((FP, L), dtype=np.float32)
    cosp[:F] = cos
    # Wl[li, p, fi, b] = Wp[fi*128+p, li*128+b]
    Wl = np.ascontiguousarray(
        Wp.reshape(FT, 128, LT, 128).transpose(2, 1, 0, 3).astype(NP_BF16))
    # per-core cos shard: cs[p, fi, n] = cosp[fi*128+p, s*CS+n]
    csr = cosp.reshape(FT, 128, N_CORES, CS).transpose(2, 1, 0, 3)
    css = [np.ascontiguousarray(csr[s].astype(NP_BF16))
           for s in range(N_CORES)]
    # xTf[mc, p, li, m] = x[mc*MW+m, li*128+p]  (replicated to all cores)
    xTf = np.ascontiguousarray(
        x.reshape(MC, MW, LT, 128).transpose(0, 3, 2, 1).astype(NP_BF16))
    return Wl, css, xTf


_out_names = ["outT"]


def _in_maps(x, W, cos_basis):
    Wl, css, xTf = _prep_inputs(x, W, cos_basis)
    return [{"Wl": Wl, "cs": css[i], "xTf": xTf} for i in range(N_CORES)]


def _post(results):
    # outT[core s][l2p, p, m] = out[m, s*CS + l2p*128 + p]
    shards = []
    for i in range(N_CORES):
        o = results[i]["outT"].astype(np.float32)  # [2, 128, B]
        shards.append(o.reshape(CS, B).T)          # [B, CS]
    return np.ascontiguousarray(np.concatenate(shards, axis=1))


def kernel(x, W, cos_basis, _trace=False, _trace_kwargs=None):
    in_maps = _in_maps(x, W, cos_basis)
    nc = _get_nc()
    res = bass_utils.run_bass_kernel_spmd(
        nc, in_maps, core_ids=list(range(N_CORES)), trace=_trace,
        **(_trace_kwargs or {}))
    out = _post(res.results)
    if _trace:
        kernel.last_result = res
    return out
